# revision 19
# baseline (speedup 1.0000x reference)
"""Bass/Trainium2 kernel for nn_DiagonalTraining (per-anti-diagonal Linear).

Math: for each anti-diagonal i of x[B,S,S] (entries x[b,r,i-r], r<=i),
apply Linear_i (weights W[i,:i+1,:i+1], bias b[i,:i+1]) to the gathered
vector and scatter back reversed. Equivalent to:
    D[b,i,j] = x[b,j,i-j] (j<=i else 0)
    out[b,i,k] = sum_j W[i,k,j] * D[b,i,j] + b[i,k]
    new_x[b,r,c] = out[b,r+c,c] if r+c < S else x[b,r,c]

Device does the einsum (memory-bound: streams the valid triangle of W);
gather/scatter/bias are tiny O(S^2) host ops.

Sharding: interleaved over diagonals — core c owns i = c, c+8, ..., c+504
(slot m holds diagonal 8m+c, padded to length L=8(m+1)). All cores run one
identical SPMD program with near-identical work; padding rows/cols of W and
D are zero by construction so results are exact.

Device layout: W is host-packed into the exact SBUF image each matmul
wants ([j-partition, (chunk, k)] per slot), so every W load is one
dma_start with multi-KB contiguous per-partition descriptors. Four
consecutive slots share one PSUM bank via tile_position col-packing
(output partitions 32t..32t+8), giving PE col-group concurrency and a
128-partition PSUM->DRAM output DMA per group.

fp8 mode: W image is fp8e4 (host-scaled by 128, well inside +-240) while
D stays bf16 (PE allows mixed non-fp32 operand dtypes); output staged as
bf16. Host divides by 128 and adds bias. Halves the dominant W stream.
"""

import sys

sys.path.insert(0, "/opt/trn_rl_repo")

import numpy as np

B = 8
S = 512
NCORES = 8
M = 64  # diagonal slots per core
LBAR = [8 * (m + 1) for m in range(M)]  # padded diagonal length per slot
NQ = [1 if m < 16 else (m // 16 + 1) for m in range(M)]  # j-chunks per slot
QOFF = np.cumsum([0] + NQ).tolist()  # chunk-column offset per slot in dt image
DTOT = QOFF[M]  # 160 chunks
# wimg column offsets: slot m's image is [128, NQ[m]*LBAR[m]]
WCUM = np.cumsum([0] + [NQ[m] * LBAR[m] for m in range(M)]).tolist()
WTOT = WCUM[M]  # 51840
SMALL_TOT = WCUM[16]  # 1088 (slots 0..15, single chunk, resident)
G = 16  # groups of 4 slots sharing a PSUM bank
LG = [32 * (g + 1) for g in range(G)]  # group output width
OCUM = np.cumsum([0] + LG).tolist()
OTOT = OCUM[G]  # 4352
WSCALE = 128.0  # fp8 host-side W scale (undone on host after)


def _build_dr_layout():
    """Layouts for fp8dr: D image columns and compact out-big columns.

    D image: m<16 plain chunks 8 cols; m>=16 DoubleRow pair blocks of 32
    cols (two 16-wide chunks: 8 real batch cols + 8 zero cols, so the
    lhsT AP [128, 2, 16] meets the DR step%16 ISA rule); odd third chunk
    (nq=3) is a 16-wide plain chunk (out partitions [0:16] like DR).

    out_big: groups 4-7 use one 1024-col block per group (two shared
    PSUM banks, slots at quarter offsets); groups 8-15 pack per-slot
    [8, L] blocks back to back.
    """
    dcol = {}
    col = 0
    for m in range(16):
        dcol[(m, 0)] = col
        col += 8
    for m in range(16, 64):
        nq = NQ[m]
        for r in range(nq // 2):
            dcol[(m, 2 * r)] = col
            col += 32
        if nq % 2:
            dcol[(m, nq - 1)] = col
            col += 16
    dtot = col  # 2432
    ocol = {}
    c = 0
    for g in range(4, 8):
        for t in range(4):
            ocol[4 * g + t] = c + (t // 2) * 512 + (t % 2) * 256
        c += 1024
    for g in range(8, 16):
        for t in range(4):
            ocol[4 * g + t] = c
            c += LBAR[4 * g + t]
    return dcol, dtot, ocol, c


DCOL, DR_DTOT, OBCOL, OBCOLS = _build_dr_layout()
OCUM_S = np.cumsum([0] + LG[0:4]).tolist()  # out_small group offsets (320)

# "fp8dr" (W+D fp8e4, DoubleRow matmuls), "fp8" (W fp8e4 + D bf16),
# "bf16" (both bf16), "fp32" (exact)
MODE = "fp8dr"

_compiled = {}


def _np_dtype(mode):
    import ml_dtypes

    if mode == "bf16":
        return np.dtype(ml_dtypes.bfloat16)
    if mode in ("fp8", "fp8dr"):
        return np.dtype(ml_dtypes.float8_e4m3)
    return np.dtype(np.float32)


def _bir_dtypes(mode):
    """(w_dtype, d_dtype, out_dtype) BIR dtypes for the mode."""
    import concourse.mybir as mybir

    return {
        "fp32": (mybir.dt.float32, mybir.dt.float32, mybir.dt.float32),
        "bf16": (mybir.dt.bfloat16, mybir.dt.bfloat16, mybir.dt.float32),
        "fp8": (mybir.dt.float8e4, mybir.dt.bfloat16, mybir.dt.bfloat16),
        "fp8dr": (mybir.dt.float8e4, mybir.dt.float8e4, mybir.dt.bfloat16),
    }[mode]


def build_program_dr():
    """fp8 DoubleRow program: PE ingests 2 W bytes/cycle on chunk pairs.

    DR constraints (walrus ISA): both operands fp8, lhsT AP [128, 2, F]
    with F*elem%16==0 and F>=16, out partitions [0:2F//2] starting at
    partition 0, no tile_position. So slots m>=16 run DR with 16-wide
    zero-padded lhsT chunks (out rows 8..15 are zeros) and per-slot (or
    pair-shared) PSUM banks; slots m<16 keep plain col-packed matmuls.
    Outputs: groups 0-3 full-width copy + [128, LG] DMA (cheap); groups
    4-15 narrow [8, *] copies (vector/gpsimd split) + compact [8, *]
    group DMAs on the sync/scalar queues.
    """
    import concourse.mybir as mybir
    import concourse.tile as tile
    from concourse import bacc

    wdt, ddt, odt = _bir_dtypes("fp8dr")
    f32 = mybir.dt.float32
    DR = mybir.MatmulPerfMode.DoubleRow

    nc = bacc.Bacc("TRN2")
    wimg = nc.dram_tensor("wimg", [128, WTOT], wdt, kind="ExternalInput")
    dt_in = nc.dram_tensor("dt", [128, DR_DTOT], ddt, kind="ExternalInput")
    out_s = nc.dram_tensor("out_s", [128, OCUM_S[4]], odt, kind="ExternalOutput")
    out_b = nc.dram_tensor("out_b", [8, OBCOLS], odt, kind="ExternalOutput")

    with tile.TileContext(nc) as tc:
        with (
            tc.tile_pool(name="dpool", bufs=1) as dpool,
            tc.tile_pool(name="wspool", bufs=1) as wspool,
            tc.tile_pool(name="wpool", bufs=6) as wpool,
            tc.tile_pool(name="opool", bufs=2) as opool,
            tc.tile_pool(name="ocpool", bufs=8) as ocpool,
            tc.tile_pool(name="psum", bufs=8, space="PSUM") as psum_pool,
        ):
            dma_engines = [nc.sync, nc.scalar]
            n_dma = 0

            def fetch_group(g):
                nonlocal n_dma
                gw = WCUM[4 * g + 4] - WCUM[4 * g]
                wtile = wpool.tile([128, 8000], wdt, tag="w")
                eng = dma_engines[n_dma % 2]
                n_dma += 1
                eng.dma_start(
                    wtile[0:128, 0:gw], wimg[:, WCUM[4 * g] : WCUM[4 * g + 4]]
                )
                return wtile

            # Tiny gating inputs first, then the W stream.
            dtall = dpool.tile([128, DR_DTOT], ddt)
            nc.sync.dma_start(dtall[:], dt_in[:, :])
            wsmall = wspool.tile([128, SMALL_TOT], wdt)
            nc.scalar.dma_start(wsmall[:], wimg[:, 0:SMALL_TOT])
            prefetched = {15: fetch_group(15), 14: fetch_group(14)}

            # GpSimd cannot read PSUM; split the narrow PSUM->SBUF copies
            # between DVE (tensor_copy) and the Activation engine (copy).
            n_copy = 0

            def psum_copy(dst, src):
                nonlocal n_copy
                if n_copy % 2 == 0:
                    nc.vector.tensor_copy(dst, src)
                else:
                    nc.scalar.copy(dst, src)
                n_copy += 1

            def dr_slot(m, ps, wtile, pcol):
                """Emit the matmul(s) for DR slot m into ps[0:16, pcol:pcol+L]."""
                L = LBAR[m]
                nq = NQ[m]
                woff = WCUM[m] - WCUM[4 * (m // 4)]
                out_ap = ps[0:16, pcol : pcol + L]
                for r in range(nq // 2):
                    rhs = wtile[
                        0:128, woff + 2 * r * L : woff + (2 * r + 2) * L
                    ].rearrange("p (two l) -> p two l", two=2)
                    c = DCOL[(m, 2 * r)]
                    lhsT = dtall[0:128, c : c + 32].rearrange(
                        "p (two z) -> p two z", two=2
                    )
                    nc.tensor.matmul(
                        out_ap,
                        lhsT=lhsT,
                        rhs=rhs,
                        start=(r == 0),
                        stop=(2 * r + 2 == nq),
                        perf_mode=DR,
                    )
                if nq % 2:
                    q = nq - 1
                    c = DCOL[(m, q)]
                    nc.tensor.matmul(
                        out_ap,
                        lhsT=dtall[0:128, c : c + 16],
                        rhs=wtile[0:128, woff + q * L : woff + (q + 1) * L],
                        start=False,
                        stop=True,
                    )

            for g in [3, 2] + list(range(G - 1, 3, -1)) + [1, 0]:
                if g >= 4:
                    wtile = prefetched.pop(g, None)
                    if wtile is None:
                        wtile = fetch_group(g)
                    oc = ocpool.tile([8, 2048], odt, tag="oc")
                    if g < 8:
                        # Two shared banks: slots (t0,t1) and (t2,t3) at
                        # col offsets 0/256 within their bank.
                        for half in range(2):
                            ps = psum_pool.tile([128, 512], f32, tag="ps")
                            for k in range(2):
                                t = 2 * half + k
                                dr_slot(4 * g + t, ps, wtile, 256 * k)
                            psum_copy(
                                oc[0:8, 512 * half : 512 * half + 512],
                                ps[0:8, 0:512],
                            )
                        gcols = 1024
                    else:
                        coff = 0
                        for t in range(4):
                            m = 4 * g + t
                            ps = psum_pool.tile([128, 512], f32, tag="ps")
                            dr_slot(m, ps, wtile, 0)
                            psum_copy(
                                oc[0:8, coff : coff + LBAR[m]],
                                ps[0:8, 0 : LBAR[m]],
                            )
                            coff += LBAR[m]
                        gcols = coff
                    ob0 = OBCOL[4 * g]
                    nc.gpsimd.dma_start(
                        out_b[:, ob0 : ob0 + gcols], oc[0:8, 0:gcols]
                    )
                else:
                    ps = psum_pool.tile([128, 512], f32, tag="ps")
                    for t in range(4):
                        m = 4 * g + t
                        L = LBAR[m]
                        nc.tensor.matmul(
                            ps[32 * t : 32 * t + B, 0:L],
                            lhsT=dtall[0:128, DCOL[(m, 0)] : DCOL[(m, 0)] + 8],
                            rhs=wsmall[0:128, WCUM[m] : WCUM[m] + L],
                            start=True,
                            stop=True,
                            tile_position=(0, 32 * t),
                        )
                    ot = opool.tile([128, 128], odt, tag="ostage")
                    nc.vector.tensor_copy(ot[0:128, 0 : LG[g]], ps[0:128, 0 : LG[g]])
                    nc.gpsimd.dma_start(
                        out_s[:, OCUM_S[g] : OCUM_S[g + 1]], ot[0:128, 0 : LG[g]]
                    )

    nc.compile()
    return nc


def build_program(mode=MODE):
    """Build the SPMD Bass program (same instructions on all 8 cores)."""
    import concourse.mybir as mybir
    import concourse.tile as tile
    from concourse import bacc

    if mode == "fp8dr":
        return build_program_dr()

    wdt, ddt, odt = _bir_dtypes(mode)
    f32 = mybir.dt.float32
    use_dr = False

    nc = bacc.Bacc("TRN2")
    wimg = nc.dram_tensor("wimg", [128, WTOT], wdt, kind="ExternalInput")
    dt_in = nc.dram_tensor("dt", [128, DTOT * B], ddt, kind="ExternalInput")
    out = nc.dram_tensor("out", [128, OTOT], odt, kind="ExternalOutput")

    with tile.TileContext(nc) as tc:
        with (
            tc.tile_pool(name="dpool", bufs=1) as dpool,
            tc.tile_pool(name="wspool", bufs=1) as wspool,
            tc.tile_pool(name="wpool", bufs=6) as wpool,
            tc.tile_pool(name="opool", bufs=4) as opool,
            tc.tile_pool(name="psum", bufs=8, space="PSUM") as psum_pool,
        ):
            dma_engines = [nc.sync, nc.scalar]
            n_dma = 0

            def fetch_group(g):
                nonlocal n_dma
                # One DMA per group: the 4 members' images are adjacent
                # in wimg, so this is a single large transfer with
                # multi-KB contiguous per-partition descriptors.
                gw = WCUM[4 * g + 4] - WCUM[4 * g]
                wtile = wpool.tile([128, 8000], wdt, tag="w")
                eng = dma_engines[n_dma % 2]
                n_dma += 1
                eng.dma_start(
                    wtile[0:128, 0:gw], wimg[:, WCUM[4 * g] : WCUM[4 * g + 4]]
                )
                return wtile

            # The tiny D / small-W images gate EVERY matmul — they must be
            # the first transfers on their queues (ahead of the big W
            # group prefetches) so the PE can start ~7us earlier.
            dtall = dpool.tile([128, DTOT * B], ddt)
            nc.sync.dma_start(dtall[:], dt_in[:, :])
            wsmall = wspool.tile([128, SMALL_TOT], wdt)
            nc.scalar.dma_start(wsmall[:], wimg[:, 0:SMALL_TOT])
            prefetched = {15: fetch_group(15), 14: fetch_group(14)}

            # Small groups 3,2 first (their W is resident — instant PE work
            # while the stream ramps), then largest-first, tiny groups last
            # (short pipeline tail).
            for g in [3, 2] + list(range(G - 1, 3, -1)) + [1, 0]:
                ps = psum_pool.tile([128, 512], f32, tag="ps")
                if g >= 4:
                    wtile = prefetched.pop(g, None)
                    if wtile is None:
                        wtile = fetch_group(g)
                for t in range(4):
                    m = 4 * g + t
                    L = LBAR[m]
                    nq = NQ[m]
                    if m < 16:
                        wt_ap = wsmall[0:128, WCUM[m] : WCUM[m] + L]
                        woff = 0
                    else:
                        woff = WCUM[m] - WCUM[4 * g]
                    q = 0
                    while q < nq:
                        pair = use_dr and (q + 1 < nq)
                        step = 2 if pair else 1
                        rhs = (
                            wt_ap
                            if m < 16
                            else wtile[
                                0:128, woff + q * L : woff + (q + step) * L
                            ]
                        )
                        lhsT = dtall[
                            0:128, (QOFF[m] + q) * B : (QOFF[m] + q + step) * B
                        ]
                        if pair:
                            rhs = rhs.rearrange("p (two l) -> p two l", two=2)
                            lhsT = lhsT.rearrange("p (two b) -> p two b", two=2)
                        nc.tensor.matmul(
                            ps[32 * t : 32 * t + B, 0:L],
                            lhsT=lhsT,
                            rhs=rhs,
                            start=(q == 0),
                            stop=(q + step == nq),
                            tile_position=(0, 32 * t),
                            perf_mode=(
                                mybir.MatmulPerfMode.DoubleRow if pair else None
                            ),
                        )
                        q += step
                ot = opool.tile([128, 512], odt, tag="ostage")
                nc.vector.tensor_copy(ot[0:128, 0 : LG[g]], ps[0:128, 0 : LG[g]])
                nc.gpsimd.dma_start(
                    out[:, OCUM[g] : OCUM[g + 1]], ot[0:128, 0 : LG[g]]
                )

    nc.compile()
    return nc


def _get_program(mode=MODE):
    if mode not in _compiled:
        _compiled[mode] = build_program(mode)
    return _compiled[mode]


def _prep_inputs(x, W, mode=MODE):
    """Host-side shard prep: gather diagonals of x, pack W SBUF images."""
    import ml_dtypes

    wnp = _np_dtype(mode)
    dnp = np.dtype(ml_dtypes.bfloat16) if mode == "fp8" else wnp
    wscale = np.float32(WSCALE) if mode in ("fp8", "fp8dr") else np.float32(1.0)

    i_idx = np.arange(S)[:, None]
    r_idx = np.arange(S)[None, :]
    cols = (i_idx - r_idx) % S
    valid = (r_idx <= i_idx)[None]
    D = np.where(valid, x[:, r_idx, cols], np.float32(0.0))  # [B, S(i), S(j)]

    in_maps = []
    for c in range(NCORES):
        Wc = W[c::8]  # [M, S(k), S(j)]
        WIMG = np.empty((128, WTOT), dtype=wnp)
        for m in range(M):
            L, nq = LBAR[m], NQ[m]
            # img[j, (q, k)] = Wc[m, k, 128q + j]
            blk = Wc[m, 0:L, 0 : 128 * nq] * wscale  # [k=L, j]
            img = blk.T.reshape(nq, 128, L).transpose(1, 0, 2).reshape(128, nq * L)
            WIMG[:, WCUM[m] : WCUM[m + 1]] = img.astype(wnp, copy=False)
        Dc = D[:, c::8, :]  # [B, M, S]
        if mode == "fp8dr":
            # v3 layout: DCOL blocks, 16-wide zero-padded chunks for m>=16
            DT = np.zeros((128, DR_DTOT), dtype=dnp)
            for m in range(M):
                for q in range(NQ[m]):
                    arr = Dc[:, m, 128 * q : 128 * (q + 1)].T  # [j=128, B]
                    if m < 16:
                        c0 = DCOL[(m, 0)]
                    elif q % 2 == 0 and (m, q) in DCOL:
                        c0 = DCOL[(m, q)]
                    elif q % 2 == 1:
                        c0 = DCOL[(m, q - 1)] + 16
                    else:
                        c0 = DCOL[(m, q)]
                    DT[:, c0 : c0 + B] = arr.astype(dnp, copy=False)
        else:
            # DT[j, qoff_m + q, b] = D[b, 8m+c, 128q+j], used chunks only
            DT = np.empty((128, DTOT * B), dtype=dnp)
            for m in range(M):
                nq = NQ[m]
                blk = Dc[:, m, 0 : 128 * nq]  # [B, 128*nq]
                arr = (
                    blk.T.reshape(nq, 128, B).transpose(1, 0, 2).reshape(128, nq * B)
                )
                DT[:, QOFF[m] * B : (QOFF[m] + nq) * B] = arr.astype(dnp, copy=False)
        in_maps.append({"wimg": WIMG, "dt": DT})
    return in_maps


def _postprocess(x, bvec, results, mode=MODE):
    """Assemble per-core outputs, undo W scale, add bias, scatter back."""
    inv_scale = (
        np.float32(1.0 / WSCALE) if mode in ("fp8", "fp8dr") else np.float32(1.0)
    )
    out_full = np.empty((B, S, S), dtype=np.float32)
    for c in range(NCORES):
        if mode == "fp8dr":
            o_s = np.asarray(results[c]["out_s"]).astype(np.float32)
            o_b = np.asarray(results[c]["out_b"]).astype(np.float32)
            for g in range(4):
                blk = o_s[:, OCUM_S[g] : OCUM_S[g + 1]].reshape(4, 32, LG[g])[:, 0:B]
                for t in range(4):
                    m = 4 * g + t
                    out_full[:, 8 * m + c, 0 : LBAR[m]] = blk[t, :, 0 : LBAR[m]]
            for m in range(16, M):
                out_full[:, 8 * m + c, 0 : LBAR[m]] = o_b[
                    :, OBCOL[m] : OBCOL[m] + LBAR[m]
                ]
            continue
        o = np.asarray(results[c]["out"]).astype(np.float32)  # [128, OTOT]
        for g in range(G):
            blk = o[:, OCUM[g] : OCUM[g + 1]].reshape(4, 32, LG[g])[:, 0:B]
            for t in range(4):
                m = 4 * g + t
                out_full[:, 8 * m + c, 0 : LBAR[m]] = blk[t, :, 0 : LBAR[m]]
    out_full *= inv_scale
    out_full += bvec[None]
    rr = np.arange(S)[:, None]
    cc = np.arange(S)[None, :]
    diag = rr + cc
    new_x = np.where(
        (diag < S)[None], out_full[:, np.minimum(diag, S - 1), cc], x
    ).astype(np.float32)
    return new_x


def kernel_run(x, W, b, mode=MODE, trace=False):
    from concourse.bass_utils import run_bass_kernel_spmd

    nc = _get_program(mode)
    in_maps = _prep_inputs(x, W, mode)
    res = run_bass_kernel_spmd(nc, in_maps, list(range(NCORES)), trace=trace)
    return _postprocess(x, b, res.results, mode), res


def kernel(x, W, b):
    out, _ = kernel_run(np.asarray(x), np.asarray(W), np.asarray(b))
    return out


# revision 20
# speedup vs baseline: 1.3462x; 1.3462x over previous
"""Bass/Trainium2 kernel for nn_DiagonalTraining (per-anti-diagonal Linear).

Math: for each anti-diagonal i of x[B,S,S] (entries x[b,r,i-r], r<=i),
apply Linear_i (weights W[i,:i+1,:i+1], bias b[i,:i+1]) to the gathered
vector and scatter back reversed. Equivalent to:
    D[b,i,j] = x[b,j,i-j] (j<=i else 0)
    out[b,i,k] = sum_j W[i,k,j] * D[b,i,j] + b[i,k]
    new_x[b,r,c] = out[b,r+c,c] if r+c < S else x[b,r,c]

Device does the einsum (memory-bound: streams the valid triangle of W);
gather/scatter/bias are tiny O(S^2) host ops.

Sharding: interleaved over diagonals — core c owns i = c, c+8, ..., c+504
(slot m holds diagonal 8m+c, padded to length L=8(m+1)). All cores run one
identical SPMD program with near-identical work; padding rows/cols of W and
D are zero by construction so results are exact.

Device layout: W is host-packed into the exact SBUF image each matmul
wants ([j-partition, (chunk, k)] per slot), so every W load is one
dma_start with multi-KB contiguous per-partition descriptors. Four
consecutive slots share one PSUM bank via tile_position col-packing
(output partitions 32t..32t+8), giving PE col-group concurrency and a
128-partition PSUM->DRAM output DMA per group.

fp8 mode: W image is fp8e4 (host-scaled by 128, well inside +-240) while
D stays bf16 (PE allows mixed non-fp32 operand dtypes); output staged as
bf16. Host divides by 128 and adds bias. Halves the dominant W stream.
"""

import sys

sys.path.insert(0, "/opt/trn_rl_repo")

import numpy as np

B = 8
S = 512
NCORES = 8
M = 64  # diagonal slots per core
LBAR = [8 * (m + 1) for m in range(M)]  # padded diagonal length per slot
NQ = [1 if m < 16 else (m // 16 + 1) for m in range(M)]  # j-chunks per slot
QOFF = np.cumsum([0] + NQ).tolist()  # chunk-column offset per slot in dt image
DTOT = QOFF[M]  # 160 chunks
# wimg column offsets: slot m's image is [128, NQ[m]*LBAR[m]]
WCUM = np.cumsum([0] + [NQ[m] * LBAR[m] for m in range(M)]).tolist()
WTOT = WCUM[M]  # 51840
SMALL_TOT = WCUM[16]  # 1088 (slots 0..15, single chunk, resident)
G = 16  # groups of 4 slots sharing a PSUM bank
LG = [32 * (g + 1) for g in range(G)]  # group output width
OCUM = np.cumsum([0] + LG).tolist()
OTOT = OCUM[G]  # 4352
WSCALE = 128.0  # fp8 host-side W scale (undone on host after)


def _build_dr_layout():
    """Layouts for fp8dr: D image columns and compact out-big columns.

    D image: m<16 plain chunks 8 cols; m>=16 DoubleRow pair blocks of 32
    cols (two 16-wide chunks: 8 real batch cols + 8 zero cols, so the
    lhsT AP [128, 2, 16] meets the DR step%16 ISA rule); odd third chunk
    (nq=3) is a 16-wide plain chunk (out partitions [0:16] like DR).

    out_big: groups 4-7 use one 1024-col block per group (two shared
    PSUM banks, slots at quarter offsets); groups 8-15 pack per-slot
    [8, L] blocks back to back.
    """
    dcol = {}
    col = 0
    for m in range(16):
        dcol[(m, 0)] = col
        col += 8
    for m in range(16, 64):
        nq = NQ[m]
        for r in range(nq // 2):
            dcol[(m, 2 * r)] = col
            col += 32
        if nq % 2:
            dcol[(m, nq - 1)] = col
            col += 16
    dtot = col  # 2432
    ocol = {}
    c = 0
    for g in range(4, 8):
        for t in range(4):
            ocol[4 * g + t] = c + (t // 2) * 512 + (t % 2) * 256
        c += 1024
    for g in range(8, 16):
        for t in range(4):
            ocol[4 * g + t] = c
            c += LBAR[4 * g + t]
    return dcol, dtot, ocol, c


DCOL, DR_DTOT, OBCOL, OBCOLS = _build_dr_layout()
OCUM_S = np.cumsum([0] + LG[0:4]).tolist()  # out_small group offsets (320)

# "fp8dr" (W+D fp8e4, DoubleRow matmuls), "fp8" (W fp8e4 + D bf16),
# "bf16" (both bf16), "fp32" (exact)
MODE = "fp8"

_compiled = {}


def _np_dtype(mode):
    import ml_dtypes

    if mode == "bf16":
        return np.dtype(ml_dtypes.bfloat16)
    if mode in ("fp8", "fp8dr"):
        return np.dtype(ml_dtypes.float8_e4m3)
    return np.dtype(np.float32)


def _bir_dtypes(mode):
    """(w_dtype, d_dtype, out_dtype) BIR dtypes for the mode."""
    import concourse.mybir as mybir

    return {
        "fp32": (mybir.dt.float32, mybir.dt.float32, mybir.dt.float32),
        "bf16": (mybir.dt.bfloat16, mybir.dt.bfloat16, mybir.dt.float32),
        "fp8": (mybir.dt.float8e4, mybir.dt.bfloat16, mybir.dt.bfloat16),
        "fp8dr": (mybir.dt.float8e4, mybir.dt.float8e4, mybir.dt.bfloat16),
    }[mode]


def build_program_dr():
    """fp8 DoubleRow program: PE ingests 2 W bytes/cycle on chunk pairs.

    DR constraints (walrus ISA): both operands fp8, lhsT AP [128, 2, F]
    with F*elem%16==0 and F>=16, out partitions [0:2F//2] starting at
    partition 0, no tile_position. So slots m>=16 run DR with 16-wide
    zero-padded lhsT chunks (out rows 8..15 are zeros) and per-slot (or
    pair-shared) PSUM banks; slots m<16 keep plain col-packed matmuls.
    Outputs: groups 0-3 full-width copy + [128, LG] DMA (cheap); groups
    4-15 narrow [8, *] copies (vector/gpsimd split) + compact [8, *]
    group DMAs on the sync/scalar queues.
    """
    import concourse.mybir as mybir
    import concourse.tile as tile
    from concourse import bacc

    wdt, ddt, odt = _bir_dtypes("fp8dr")
    f32 = mybir.dt.float32
    DR = mybir.MatmulPerfMode.DoubleRow

    nc = bacc.Bacc("TRN2")
    wimg = nc.dram_tensor("wimg", [128, WTOT], wdt, kind="ExternalInput")
    dt_in = nc.dram_tensor("dt", [128, DR_DTOT], ddt, kind="ExternalInput")
    out_s = nc.dram_tensor("out_s", [128, OCUM_S[4]], odt, kind="ExternalOutput")
    out_b = nc.dram_tensor("out_b", [8, OBCOLS], odt, kind="ExternalOutput")

    with tile.TileContext(nc) as tc:
        with (
            tc.tile_pool(name="dpool", bufs=1) as dpool,
            tc.tile_pool(name="wspool", bufs=1) as wspool,
            tc.tile_pool(name="wpool", bufs=6) as wpool,
            tc.tile_pool(name="opool", bufs=2) as opool,
            tc.tile_pool(name="ocpool", bufs=8) as ocpool,
            tc.tile_pool(name="psum", bufs=8, space="PSUM") as psum_pool,
        ):
            dma_engines = [nc.sync, nc.scalar]
            n_dma = 0

            def fetch_group(g):
                nonlocal n_dma
                gw = WCUM[4 * g + 4] - WCUM[4 * g]
                wtile = wpool.tile([128, 8000], wdt, tag="w")
                eng = dma_engines[n_dma % 2]
                n_dma += 1
                eng.dma_start(
                    wtile[0:128, 0:gw], wimg[:, WCUM[4 * g] : WCUM[4 * g + 4]]
                )
                return wtile

            # Tiny gating inputs first, then the W stream.
            dtall = dpool.tile([128, DR_DTOT], ddt)
            nc.sync.dma_start(dtall[:], dt_in[:, :])
            wsmall = wspool.tile([128, SMALL_TOT], wdt)
            nc.scalar.dma_start(wsmall[:], wimg[:, 0:SMALL_TOT])
            prefetched = {15: fetch_group(15), 14: fetch_group(14)}

            # GpSimd cannot read PSUM; split the narrow PSUM->SBUF copies
            # between DVE (tensor_copy) and the Activation engine (copy).
            n_copy = 0

            def psum_copy(dst, src):
                nonlocal n_copy
                if n_copy % 2 == 0:
                    nc.vector.tensor_copy(dst, src)
                else:
                    nc.scalar.copy(dst, src)
                n_copy += 1

            def dr_slot(m, ps, wtile, pcol):
                """Emit the matmul(s) for DR slot m into ps[0:16, pcol:pcol+L]."""
                L = LBAR[m]
                nq = NQ[m]
                woff = WCUM[m] - WCUM[4 * (m // 4)]
                out_ap = ps[0:16, pcol : pcol + L]
                for r in range(nq // 2):
                    rhs = wtile[
                        0:128, woff + 2 * r * L : woff + (2 * r + 2) * L
                    ].rearrange("p (two l) -> p two l", two=2)
                    c = DCOL[(m, 2 * r)]
                    lhsT = dtall[0:128, c : c + 32].rearrange(
                        "p (two z) -> p two z", two=2
                    )
                    nc.tensor.matmul(
                        out_ap,
                        lhsT=lhsT,
                        rhs=rhs,
                        start=(r == 0),
                        stop=(2 * r + 2 == nq),
                        perf_mode=DR,
                    )
                if nq % 2:
                    q = nq - 1
                    c = DCOL[(m, q)]
                    nc.tensor.matmul(
                        out_ap,
                        lhsT=dtall[0:128, c : c + 16],
                        rhs=wtile[0:128, woff + q * L : woff + (q + 1) * L],
                        start=False,
                        stop=True,
                    )

            for g in [3, 2] + list(range(G - 1, 3, -1)) + [1, 0]:
                if g >= 4:
                    wtile = prefetched.pop(g, None)
                    if wtile is None:
                        wtile = fetch_group(g)
                    oc = ocpool.tile([8, 2048], odt, tag="oc")
                    if g < 8:
                        # Two shared banks: slots (t0,t1) and (t2,t3) at
                        # col offsets 0/256 within their bank.
                        for half in range(2):
                            ps = psum_pool.tile([128, 512], f32, tag="ps")
                            for k in range(2):
                                t = 2 * half + k
                                dr_slot(4 * g + t, ps, wtile, 256 * k)
                            psum_copy(
                                oc[0:8, 512 * half : 512 * half + 512],
                                ps[0:8, 0:512],
                            )
                        gcols = 1024
                    else:
                        coff = 0
                        for t in range(4):
                            m = 4 * g + t
                            ps = psum_pool.tile([128, 512], f32, tag="ps")
                            dr_slot(m, ps, wtile, 0)
                            psum_copy(
                                oc[0:8, coff : coff + LBAR[m]],
                                ps[0:8, 0 : LBAR[m]],
                            )
                            coff += LBAR[m]
                        gcols = coff
                    ob0 = OBCOL[4 * g]
                    nc.gpsimd.dma_start(
                        out_b[:, ob0 : ob0 + gcols], oc[0:8, 0:gcols]
                    )
                else:
                    ps = psum_pool.tile([128, 512], f32, tag="ps")
                    for t in range(4):
                        m = 4 * g + t
                        L = LBAR[m]
                        nc.tensor.matmul(
                            ps[32 * t : 32 * t + B, 0:L],
                            lhsT=dtall[0:128, DCOL[(m, 0)] : DCOL[(m, 0)] + 8],
                            rhs=wsmall[0:128, WCUM[m] : WCUM[m] + L],
                            start=True,
                            stop=True,
                            tile_position=(0, 32 * t),
                        )
                    ot = opool.tile([128, 128], odt, tag="ostage")
                    nc.vector.tensor_copy(ot[0:128, 0 : LG[g]], ps[0:128, 0 : LG[g]])
                    nc.gpsimd.dma_start(
                        out_s[:, OCUM_S[g] : OCUM_S[g + 1]], ot[0:128, 0 : LG[g]]
                    )

    nc.compile()
    return nc


def build_program(mode=MODE):
    """Build the SPMD Bass program (same instructions on all 8 cores)."""
    import concourse.mybir as mybir
    import concourse.tile as tile
    from concourse import bacc

    if mode == "fp8dr":
        return build_program_dr()

    wdt, ddt, odt = _bir_dtypes(mode)
    f32 = mybir.dt.float32
    use_dr = False

    nc = bacc.Bacc("TRN2")
    wimg = nc.dram_tensor("wimg", [128, WTOT], wdt, kind="ExternalInput")
    dt_in = nc.dram_tensor("dt", [128, DTOT * B], ddt, kind="ExternalInput")
    out = nc.dram_tensor("out", [128, OTOT], odt, kind="ExternalOutput")

    with tile.TileContext(nc) as tc:
        with (
            tc.tile_pool(name="dpool", bufs=1) as dpool,
            tc.tile_pool(name="wspool", bufs=1) as wspool,
            tc.tile_pool(name="wpool", bufs=6) as wpool,
            tc.tile_pool(name="opool", bufs=4) as opool,
            tc.tile_pool(name="psum", bufs=8, space="PSUM") as psum_pool,
        ):
            dma_engines = [nc.sync, nc.scalar]
            n_dma = 0

            def fetch_group(g):
                nonlocal n_dma
                # One DMA per group: the 4 members' images are adjacent
                # in wimg, so this is a single large transfer with
                # multi-KB contiguous per-partition descriptors.
                gw = WCUM[4 * g + 4] - WCUM[4 * g]
                wtile = wpool.tile([128, 8000], wdt, tag="w")
                eng = dma_engines[n_dma % 2]
                n_dma += 1
                eng.dma_start(
                    wtile[0:128, 0:gw], wimg[:, WCUM[4 * g] : WCUM[4 * g + 4]]
                )
                return wtile

            # Tiny gating inputs first (they gate every matmul), then the
            # W stream smallest-group-first: the PE starts on the resident
            # small groups at ~9us and the 4-way col-packed PE outruns the
            # stream thereafter, so every group's matmuls fire right after
            # its transfer lands. dt goes in two pieces so the m<16 slice
            # lands in ~0.1us.
            dtall = dpool.tile([128, DTOT * B], ddt)
            dsplit = QOFF[16] * B
            nc.sync.dma_start(dtall[:, 0:dsplit], dt_in[:, 0:dsplit])
            wsmall = wspool.tile([128, SMALL_TOT], wdt)
            nc.scalar.dma_start(wsmall[:], wimg[:, 0:SMALL_TOT])
            nc.sync.dma_start(dtall[:, dsplit:], dt_in[:, dsplit:])
            n_dma = 1  # first in-loop fetch (g4) goes to scalar

            for g in [3, 2, 1, 0] + list(range(4, G)):
                ps = psum_pool.tile([128, 512], f32, tag="ps")
                if g >= 4:
                    wtile = fetch_group(g)
                for t in range(4):
                    m = 4 * g + t
                    L = LBAR[m]
                    nq = NQ[m]
                    if m < 16:
                        wt_ap = wsmall[0:128, WCUM[m] : WCUM[m] + L]
                        woff = 0
                    else:
                        woff = WCUM[m] - WCUM[4 * g]
                    q = 0
                    while q < nq:
                        pair = use_dr and (q + 1 < nq)
                        step = 2 if pair else 1
                        rhs = (
                            wt_ap
                            if m < 16
                            else wtile[
                                0:128, woff + q * L : woff + (q + step) * L
                            ]
                        )
                        lhsT = dtall[
                            0:128, (QOFF[m] + q) * B : (QOFF[m] + q + step) * B
                        ]
                        if pair:
                            rhs = rhs.rearrange("p (two l) -> p two l", two=2)
                            lhsT = lhsT.rearrange("p (two b) -> p two b", two=2)
                        nc.tensor.matmul(
                            ps[32 * t : 32 * t + B, 0:L],
                            lhsT=lhsT,
                            rhs=rhs,
                            start=(q == 0),
                            stop=(q + step == nq),
                            tile_position=(0, 32 * t),
                            perf_mode=(
                                mybir.MatmulPerfMode.DoubleRow if pair else None
                            ),
                        )
                        q += step
                ot = opool.tile([128, 512], odt, tag="ostage")
                if g % 2 == 0:
                    nc.vector.tensor_copy(ot[0:128, 0 : LG[g]], ps[0:128, 0 : LG[g]])
                else:
                    nc.scalar.copy(ot[0:128, 0 : LG[g]], ps[0:128, 0 : LG[g]])
                nc.gpsimd.dma_start(
                    out[:, OCUM[g] : OCUM[g + 1]], ot[0:128, 0 : LG[g]]
                )

    nc.compile()
    return nc


def _get_program(mode=MODE):
    if mode not in _compiled:
        _compiled[mode] = build_program(mode)
    return _compiled[mode]


def _prep_inputs(x, W, mode=MODE):
    """Host-side shard prep: gather diagonals of x, pack W SBUF images."""
    import ml_dtypes

    wnp = _np_dtype(mode)
    dnp = np.dtype(ml_dtypes.bfloat16) if mode == "fp8" else wnp
    wscale = np.float32(WSCALE) if mode in ("fp8", "fp8dr") else np.float32(1.0)

    i_idx = np.arange(S)[:, None]
    r_idx = np.arange(S)[None, :]
    cols = (i_idx - r_idx) % S
    valid = (r_idx <= i_idx)[None]
    D = np.where(valid, x[:, r_idx, cols], np.float32(0.0))  # [B, S(i), S(j)]

    in_maps = []
    for c in range(NCORES):
        Wc = W[c::8]  # [M, S(k), S(j)]
        WIMG = np.empty((128, WTOT), dtype=wnp)
        for m in range(M):
            L, nq = LBAR[m], NQ[m]
            # img[j, (q, k)] = Wc[m, k, 128q + j]
            blk = Wc[m, 0:L, 0 : 128 * nq] * wscale  # [k=L, j]
            img = blk.T.reshape(nq, 128, L).transpose(1, 0, 2).reshape(128, nq * L)
            WIMG[:, WCUM[m] : WCUM[m + 1]] = img.astype(wnp, copy=False)
        Dc = D[:, c::8, :]  # [B, M, S]
        if mode == "fp8dr":
            # v3 layout: DCOL blocks, 16-wide zero-padded chunks for m>=16
            DT = np.zeros((128, DR_DTOT), dtype=dnp)
            for m in range(M):
                for q in range(NQ[m]):
                    arr = Dc[:, m, 128 * q : 128 * (q + 1)].T  # [j=128, B]
                    if m < 16:
                        c0 = DCOL[(m, 0)]
                    elif q % 2 == 0 and (m, q) in DCOL:
                        c0 = DCOL[(m, q)]
                    elif q % 2 == 1:
                        c0 = DCOL[(m, q - 1)] + 16
                    else:
                        c0 = DCOL[(m, q)]
                    DT[:, c0 : c0 + B] = arr.astype(dnp, copy=False)
        else:
            # DT[j, qoff_m + q, b] = D[b, 8m+c, 128q+j], used chunks only
            DT = np.empty((128, DTOT * B), dtype=dnp)
            for m in range(M):
                nq = NQ[m]
                blk = Dc[:, m, 0 : 128 * nq]  # [B, 128*nq]
                arr = (
                    blk.T.reshape(nq, 128, B).transpose(1, 0, 2).reshape(128, nq * B)
                )
                DT[:, QOFF[m] * B : (QOFF[m] + nq) * B] = arr.astype(dnp, copy=False)
        in_maps.append({"wimg": WIMG, "dt": DT})
    return in_maps


def _postprocess(x, bvec, results, mode=MODE):
    """Assemble per-core outputs, undo W scale, add bias, scatter back."""
    inv_scale = (
        np.float32(1.0 / WSCALE) if mode in ("fp8", "fp8dr") else np.float32(1.0)
    )
    out_full = np.empty((B, S, S), dtype=np.float32)
    for c in range(NCORES):
        if mode == "fp8dr":
            o_s = np.asarray(results[c]["out_s"]).astype(np.float32)
            o_b = np.asarray(results[c]["out_b"]).astype(np.float32)
            for g in range(4):
                blk = o_s[:, OCUM_S[g] : OCUM_S[g + 1]].reshape(4, 32, LG[g])[:, 0:B]
                for t in range(4):
                    m = 4 * g + t
                    out_full[:, 8 * m + c, 0 : LBAR[m]] = blk[t, :, 0 : LBAR[m]]
            for m in range(16, M):
                out_full[:, 8 * m + c, 0 : LBAR[m]] = o_b[
                    :, OBCOL[m] : OBCOL[m] + LBAR[m]
                ]
            continue
        o = np.asarray(results[c]["out"]).astype(np.float32)  # [128, OTOT]
        for g in range(G):
            blk = o[:, OCUM[g] : OCUM[g + 1]].reshape(4, 32, LG[g])[:, 0:B]
            for t in range(4):
                m = 4 * g + t
                out_full[:, 8 * m + c, 0 : LBAR[m]] = blk[t, :, 0 : LBAR[m]]
    out_full *= inv_scale
    out_full += bvec[None]
    rr = np.arange(S)[:, None]
    cc = np.arange(S)[None, :]
    diag = rr + cc
    new_x = np.where(
        (diag < S)[None], out_full[:, np.minimum(diag, S - 1), cc], x
    ).astype(np.float32)
    return new_x


def kernel_run(x, W, b, mode=MODE, trace=False):
    from concourse.bass_utils import run_bass_kernel_spmd

    nc = _get_program(mode)
    in_maps = _prep_inputs(x, W, mode)
    res = run_bass_kernel_spmd(nc, in_maps, list(range(NCORES)), trace=trace)
    return _postprocess(x, b, res.results, mode), res


def kernel(x, W, b):
    out, _ = kernel_run(np.asarray(x), np.asarray(W), np.asarray(b))
    return out
